# revision 19
# baseline (speedup 1.0000x reference)
"""Bi-directional attention fusion kernel for Trainium2 (8 NeuronCores).

Computes, per batch b (fully data-parallel, one batch per core):
    S       = g @ l.T                                  # [N, N]
    out     = 0.5 * (softmax_rows(S) @ l + softmax_rows(S.T) @ g)

Key algebraic trick: with a *global* stabilization constant c,
    E = exp(S - c)   (layout [g, l])
    F = exp(S.T - c) = E.T  (layout [l, g])
and the two attention terms become plain matmuls where E and F are the
pre-transposed (lhsT) operands directly:
    term1[g, d] = sum_l E[g, l] * l_emb[l, d]  =  (F as lhsT).T @ [l_emb | 1]
    term2[l, d] = sum_g E[g, l] * g_emb[g, d]  =  (E as lhsT).T @ [g_emb | 1]
The appended ones-column yields the softmax normalizers for free.

Schedule per core:
    A: S row-tiles via fp32r matmuls (1 cyc/row), exp via ACT -> E (bf16).
       Kept as one unbroken matmul stream (interleaving transposes into it
       measurably slows the fp32r pipeline).
    B: bulk PE transposes E -> F (bf16, block-transposed F layout)
    C: AV matmuls (bf16), term1/term2 interleaved per k so consecutive
       matmuls never target the same PSUM bank; normalize/average epilogue
"""

import numpy as np
import ml_dtypes

import concourse.bass as bass
import concourse.tile as tile
from concourse import bacc, mybir
from concourse.bass_utils import run_bass_kernel_spmd

BF16 = mybir.dt.bfloat16
F32 = mybir.dt.float32
F32R = mybir.dt.float32r

B = 8
N = 2048
D = 768
C_STAB = 116.0  # global softmax shift; logits are N(0, sqrt(768)) -> max ~111

N_CORES = 8
N_WARMUP_MM = 190  # dummy matmuls holding the PE HAM warm during input DMA
_ts = bass.ts


def build_nc(n=N, d=D, c_stab=C_STAB):
    """Build the per-core Bass program (identical on all cores)."""
    nt = n // 128  # row tiles
    kd = d // 128  # contraction tiles over the embedding dim
    dp1 = d + 1  # ones column appended
    nh = n // 2
    nchunk = n // 512  # 512-wide column chunks of S / lt

    nc = bacc.Bacc(None, target_bir_lowering=False)

    # host-pretiled inputs (every DMA is contiguous per partition row):
    #   gtt[gi][p, k, c] = g[gi*128+c, k*128+p]
    #   ltt[j][p, k, c]  = l[j*512+c, k*128+p]
    gt_d = nc.dram_tensor("gtt", [nt, 128, kd, 128], F32R, kind="ExternalInput")
    lt_d = nc.dram_tensor("ltt", [nchunk, 128, kd, 512], F32R, kind="ExternalInput")
    gn_d = nc.dram_tensor("gn", [n, dp1], BF16, kind="ExternalInput")
    ln_d = nc.dram_tensor("ln", [n, dp1], BF16, kind="ExternalInput")
    id_d = nc.dram_tensor("ident", [128, 128], BF16, kind="ExternalInput")
    out_d = nc.dram_tensor("out", [n, d], F32, kind="ExternalOutput")

    gn_r = gn_d[:].rearrange("(i p) d -> p i d", p=128)  # [128, nt, dp1]
    ln_r = ln_d[:].rearrange("(i p) d -> p i d", p=128)

    with tile.TileContext(nc) as tc:
        with (
            tc.tile_pool(name="const", bufs=1) as const_pool,
            tc.tile_pool(name="e", bufs=1) as e_pool,
            tc.tile_pool(name="f", bufs=1) as f_pool,
            tc.tile_pool(name="ln_in", bufs=1) as ln_pool,
        ):
            ident = const_pool.tile([128, 128], BF16)
            nc.sync.dma_start(ident[:], id_d[:])
            biasc = const_pool.tile([128, 1], F32)
            nc.vector.memset(biasc[:], -c_stab)
            e_sb = e_pool.tile([128, nt, n], BF16)  # E[g, l]
            f_sb = f_pool.tile([128, nt, n], BF16)  # F[:, gi, lj*128] blocks
            ln_sb = ln_pool.tile([128, nt, dp1], BF16)

            # ---- Stage A: E = exp(g @ l.T - c) ----
            with (
                tc.tile_pool(name="a_lt", bufs=1) as lt_pool,
                tc.tile_pool(name="a_gt", bufs=2) as gt_pool,
            ):
                lt_sb = lt_pool.tile([128, kd, n], F32R)
                nc.sync.dma_start(lt_sb[:, :, 0:512], lt_d[0])

                with tc.tile_pool(name="ps_a", bufs=2, space="PSUM") as ps_a:
                    # HAM warm-up on a ps-tag slot (results are dummies)
                    wps = ps_a.tile([128, n], F32, tag="ps")
                    for _ in range(N_WARMUP_MM):
                        nc.tensor.matmul(wps[:, 0:128], ident[:], ident[:])

                    for gi in range(nt):
                        gts = gt_pool.tile([128, kd, 128], F32R, tag="gts")
                        nc.sync.dma_start(gts[:], gt_d[gi])
                        if gi == 0:
                            for j in range(1, nchunk):
                                nc.sync.dma_start(
                                    lt_sb[:, :, _ts(j, 512)], lt_d[j]
                                )
                        if gi == 1:
                            # ln is needed first in stage C; load it early
                            for cc in range(0, nt, 4):
                                nc.sync.dma_start(
                                    ln_sb[:, cc : cc + 4, :],
                                    ln_r[:, cc : cc + 4, :],
                                )
                        ps = ps_a.tile([128, n], F32, tag="ps")
                        if gi == 0:
                            # chunk-major while lt streams in
                            for j in range(nchunk):
                                for k in range(kd):
                                    nc.tensor.matmul(
                                        ps[:, _ts(j, 512)],
                                        gts[:, k, :],
                                        lt_sb[:, k, _ts(j, 512)],
                                        start=(k == 0),
                                        stop=(k == kd - 1),
                                    )
                        else:
                            # k-major: each fp32r weight tile loads once and
                            # serves all 4 column chunks (4x fewer LDWEIGHTS)
                            for k in range(kd):
                                for j in range(nchunk):
                                    nc.tensor.matmul(
                                        ps[:, _ts(j, 512)],
                                        gts[:, k, :],
                                        lt_sb[:, k, _ts(j, 512)],
                                        start=(k == 0),
                                        stop=(k == kd - 1),
                                    )
                        for j in range(nchunk):
                            nc.scalar.activation(
                                e_sb[:, gi, _ts(j, 512)],
                                ps[:, _ts(j, 512)],
                                mybir.ActivationFunctionType.Exp,
                                bias=biasc[:],
                            )

                    # ---- Stage B: bulk transposes F = E.T ----
                    # F block (gi, lj) = transpose of e_sb[:, gi, lj*128:+128],
                    # stored at f_sb[:, gi, lj*128:+128]. pb tiles share the
                    # "ps" tag so there is no PSUM pool-transition barrier.
                    for gi in range(nt):
                        pb = ps_a.tile([128, n], BF16, tag="ps")
                        for lj in range(nt):
                            nc.tensor.transpose(
                                pb[:, _ts(lj, 128)],
                                e_sb[:, gi, _ts(lj, 128)],
                                ident[:],
                            )
                        nc.scalar.copy(f_sb[:, gi, 0:nh], pb[:, 0:nh])
                        nc.vector.tensor_copy(f_sb[:, gi, nh:n], pb[:, nh:n])


            # ---- Stage C: AV matmuls + normalize ----
            with tc.tile_pool(name="c_gn", bufs=1) as gn_pool:
                gn_sb = gn_pool.tile([128, nt, dp1], BF16)
                for cc in range(0, nt, 4):
                    nc.sync.dma_start(
                        gn_sb[:, cc : cc + 4, :], gn_r[:, cc : cc + 4, :]
                    )
                with (
                    tc.tile_pool(name="c_ps", bufs=2, space="PSUM") as ps_c,
                    tc.tile_pool(name="c_out", bufs=3) as out_pool,
                    tc.tile_pool(name="c_tmp", bufs=3) as tmp_pool,
                    tc.tile_pool(name="c_small", bufs=4) as small_pool,
                ):
                    av_chunks = [
                        (c0, min(c0 + 512, dp1)) for c0 in range(0, dp1, 512)
                    ]
                    for i in range(nt):
                        ps1 = ps_c.tile([128, dp1], F32, tag="ps1")
                        ps2 = ps_c.tile([128, dp1], F32, tag="ps2")
                        for k in range(nt):
                            # F block (gi=i, lj=k) is the term1 lhsT
                            lhs_f = f_sb[:, i, _ts(k, 128)]
                            lhs_e = e_sb[:, k, _ts(i, 128)]
                            st = k == 0
                            sp = k == nt - 1
                            for c0, c1 in av_chunks:
                                nc.tensor.matmul(
                                    ps1[:, c0:c1], lhs_f, ln_sb[:, k, c0:c1],
                                    start=st, stop=sp,
                                )
                            for c0, c1 in av_chunks:
                                nc.tensor.matmul(
                                    ps2[:, c0:c1], lhs_e, gn_sb[:, k, c0:c1],
                                    start=st, stop=sp,
                                )
                        # epilogue: out = 0.5*(ps1[:, :d]/Z1 + ps2[:, :d]/Z2)
                        r1 = small_pool.tile([128, 1], F32, tag="r1")
                        r2 = small_pool.tile([128, 1], F32, tag="r2")
                        nc.vector.reciprocal(r1[:], ps1[:, d:dp1])
                        nc.vector.reciprocal(r2[:], ps2[:, d:dp1])
                        nc.vector.tensor_scalar_mul(r1[:], r1[:], 0.5)
                        nc.vector.tensor_scalar_mul(r2[:], r2[:], 0.5)
                        t1 = tmp_pool.tile([128, d], F32)
                        nc.scalar.activation(
                            t1[:], ps1[:, 0:d],
                            mybir.ActivationFunctionType.Copy,
                            scale=r1[:],
                        )
                        out_t = out_pool.tile([128, d], F32)
                        nc.vector.scalar_tensor_tensor(
                            out_t[:], ps2[:, 0:d], r2[:], t1[:],
                            op0=mybir.AluOpType.mult,
                            op1=mybir.AluOpType.add,
                        )
                        nc.sync.dma_start(out_d[_ts(i, 128), :], out_t[:])

    nc.compile()
    return nc


_NC_CACHE = {}


def get_nc(n=N, d=D):
    key = (n, d)
    if key not in _NC_CACHE:
        _NC_CACHE[key] = build_nc(n, d)
    return _NC_CACHE[key]


def host_prep(global_embedding, local_embedding):
    """Build the 8 per-core input maps from full [B, N, D] fp32 inputs."""
    g = np.asarray(global_embedding, dtype=np.float32)
    l = np.asarray(local_embedding, dtype=np.float32)
    b, n, d = g.shape
    nt, kd, nchunk = n // 128, d // 128, n // 512
    ident = np.eye(128, dtype=ml_dtypes.bfloat16)
    ones = np.ones((n, 1), np.float32)
    in_maps = []
    for i in range(b):
        # gtt[gi, p, k, c] = g[i][gi*128 + c, k*128 + p]
        gtt = np.ascontiguousarray(
            g[i].reshape(nt, 128, kd, 128).transpose(0, 3, 2, 1)
        )
        # ltt[j, p, k, c] = l[i][j*512 + c, k*128 + p]
        ltt = np.ascontiguousarray(
            l[i].reshape(nchunk, 512, kd, 128).transpose(0, 3, 2, 1)
        )
        in_maps.append(
            {
                "gtt": gtt,
                "ltt": ltt,
                "gn": np.concatenate([g[i], ones], axis=1).astype(
                    ml_dtypes.bfloat16
                ),
                "ln": np.concatenate([l[i], ones], axis=1).astype(
                    ml_dtypes.bfloat16
                ),
                "ident": ident,
            }
        )
    return in_maps


def kernel(global_embedding, local_embedding):
    g = np.asarray(global_embedding, dtype=np.float32)
    b, n, d = g.shape
    nc = get_nc(n, d)
    in_maps = host_prep(global_embedding, local_embedding)
    res = run_bass_kernel_spmd(nc, in_maps, list(range(N_CORES)))
    return np.stack([res.results[i]["out"] for i in range(b)]).astype(np.float32)


# revision 20
# speedup vs baseline: 1.0510x; 1.0510x over previous
"""Bi-directional attention fusion kernel for Trainium2 (8 NeuronCores).

Computes, per batch b (fully data-parallel, one batch per core):
    S       = g @ l.T                                  # [N, N]
    out     = 0.5 * (softmax_rows(S) @ l + softmax_rows(S.T) @ g)

Key algebraic trick: with a *global* stabilization constant c,
    E = exp(S - c)   (layout [g, l])
    F = exp(S.T - c) = E.T  (layout [l, g])
and the two attention terms become plain matmuls where E and F are the
pre-transposed (lhsT) operands directly:
    term1[g, d] = sum_l E[g, l] * l_emb[l, d]  =  (F as lhsT).T @ [l_emb | 1]
    term2[l, d] = sum_g E[g, l] * g_emb[g, d]  =  (E as lhsT).T @ [g_emb | 1]
The appended ones-column yields the softmax normalizers for free.

Schedule per core:
    A: S row-tiles via fp32r matmuls (1 cyc/row), exp via ACT -> E (bf16).
       Kept as one unbroken matmul stream (interleaving transposes into it
       measurably slows the fp32r pipeline).
    B: bulk PE transposes E -> F (bf16, block-transposed F layout)
    C: AV matmuls (bf16), term1/term2 interleaved per k so consecutive
       matmuls never target the same PSUM bank; normalize/average epilogue
"""

import numpy as np
import ml_dtypes

import concourse.bass as bass
import concourse.tile as tile
from concourse import bacc, mybir
from concourse.bass_utils import run_bass_kernel_spmd

BF16 = mybir.dt.bfloat16
F32 = mybir.dt.float32
F32R = mybir.dt.float32r

B = 8
N = 2048
D = 768
C_STAB = 116.0  # global softmax shift; logits are N(0, sqrt(768)) -> max ~111

N_CORES = 8
N_WARMUP_MM = 190  # dummy matmuls holding the PE HAM warm during input DMA
_ts = bass.ts


def build_nc(n=N, d=D, c_stab=C_STAB):
    """Build the per-core Bass program (identical on all cores)."""
    nt = n // 128  # row tiles
    kd = d // 128  # contraction tiles over the embedding dim
    dp1 = d + 1  # ones column appended
    nh = n // 2
    nchunk = n // 512  # 512-wide column chunks of S / lt

    nc = bacc.Bacc(None, target_bir_lowering=False)

    # host-pretiled inputs (every DMA is contiguous per partition row):
    #   gtt[gi][p, k, c] = g[gi*128+c, k*128+p]
    #   ltt[j][p, k, c]  = l[j*512+c, k*128+p]
    gt_d = nc.dram_tensor("gtt", [nt, 128, kd, 128], F32R, kind="ExternalInput")
    lt_d = nc.dram_tensor("ltt", [nchunk, 128, kd, 512], F32R, kind="ExternalInput")
    gn_d = nc.dram_tensor("gn", [n, dp1], BF16, kind="ExternalInput")
    ln_d = nc.dram_tensor("ln", [n, dp1], BF16, kind="ExternalInput")
    id_d = nc.dram_tensor("ident", [128, 128], BF16, kind="ExternalInput")
    out_d = nc.dram_tensor("out", [n, d], F32, kind="ExternalOutput")

    gn_r = gn_d[:].rearrange("(i p) d -> p i d", p=128)  # [128, nt, dp1]
    ln_r = ln_d[:].rearrange("(i p) d -> p i d", p=128)

    with tile.TileContext(nc) as tc:
        with (
            tc.tile_pool(name="const", bufs=1) as const_pool,
            tc.tile_pool(name="e", bufs=1) as e_pool,
            tc.tile_pool(name="f", bufs=1) as f_pool,
            tc.tile_pool(name="ln_in", bufs=1) as ln_pool,
        ):
            ident = const_pool.tile([128, 128], BF16)
            nc.sync.dma_start(ident[:], id_d[:])
            biasc = const_pool.tile([128, 1], F32)
            nc.vector.memset(biasc[:], -c_stab)
            e_sb = e_pool.tile([128, nt, n], BF16)  # E[g, l]
            f_sb = f_pool.tile([128, nt, n], BF16)  # F[:, gi, lj*128] blocks
            ln_sb = ln_pool.tile([128, nt, dp1], BF16)

            # ---- Stage A: E = exp(g @ l.T - c) ----
            with (
                tc.tile_pool(name="a_lt", bufs=1) as lt_pool,
                tc.tile_pool(name="a_gt", bufs=2) as gt_pool,
            ):
                lt_sb = lt_pool.tile([128, kd, n], F32R)
                nc.sync.dma_start(lt_sb[:, :, 0:512], lt_d[0])

                with tc.tile_pool(name="ps_a", bufs=2, space="PSUM") as ps_a:
                    # HAM warm-up on a ps-tag slot (results are dummies)
                    wps = ps_a.tile([128, n], F32, tag="ps")
                    for _ in range(N_WARMUP_MM):
                        nc.tensor.matmul(wps[:, 0:128], ident[:], ident[:])

                    for gi in range(nt):
                        gts = gt_pool.tile([128, kd, 128], F32R, tag="gts")
                        nc.sync.dma_start(gts[:], gt_d[gi])
                        if gi == 0:
                            for j in range(1, nchunk):
                                nc.sync.dma_start(
                                    lt_sb[:, :, _ts(j, 512)], lt_d[j]
                                )
                        if gi == 1:
                            # ln is needed first in stage C; load it early
                            for cc in range(0, nt, 4):
                                nc.sync.dma_start(
                                    ln_sb[:, cc : cc + 4, :],
                                    ln_r[:, cc : cc + 4, :],
                                )
                        ps = ps_a.tile([128, n], F32, tag="ps")
                        if gi == 0:
                            # chunk-major while lt streams in
                            for j in range(nchunk):
                                for k in range(kd):
                                    nc.tensor.matmul(
                                        ps[:, _ts(j, 512)],
                                        gts[:, k, :],
                                        lt_sb[:, k, _ts(j, 512)],
                                        start=(k == 0),
                                        stop=(k == kd - 1),
                                    )
                        else:
                            # k-major: each fp32r weight tile loads once and
                            # serves all 4 column chunks (4x fewer LDWEIGHTS)
                            for k in range(kd):
                                for j in range(nchunk):
                                    nc.tensor.matmul(
                                        ps[:, _ts(j, 512)],
                                        gts[:, k, :],
                                        lt_sb[:, k, _ts(j, 512)],
                                        start=(k == 0),
                                        stop=(k == kd - 1),
                                    )
                        for j in range(nchunk):
                            nc.scalar.activation(
                                e_sb[:, gi, _ts(j, 512)],
                                ps[:, _ts(j, 512)],
                                mybir.ActivationFunctionType.Exp,
                                bias=biasc[:],
                            )




            # ---- Stage B: bulk transposes F = E.T ----
            # F block (gi, lj) lives at f_sb[:, gi, lj*128:+128] and equals
            # the transpose of e_sb[:, gi, lj*128:+128].
            with tc.tile_pool(name="b_ps", bufs=3, space="PSUM") as ps_b:
                for gi in range(nt):
                    pb = ps_b.tile([128, n], BF16)
                    for lj in range(nt):
                        nc.tensor.transpose(
                            pb[:, _ts(lj, 128)],
                            e_sb[:, gi, _ts(lj, 128)],
                            ident[:],
                        )
                    nc.scalar.copy(f_sb[:, gi, 0:nh], pb[:, 0:nh])
                    nc.vector.tensor_copy(f_sb[:, gi, nh:n], pb[:, nh:n])

            # ---- Stage C: AV matmuls + normalize ----
            with tc.tile_pool(name="c_gn", bufs=1) as gn_pool:
                gn_sb = gn_pool.tile([128, nt, dp1], BF16)
                for cc in range(0, nt, 4):
                    nc.sync.dma_start(
                        gn_sb[:, cc : cc + 4, :], gn_r[:, cc : cc + 4, :]
                    )
                with (
                    tc.tile_pool(name="c_ps", bufs=2, space="PSUM") as ps_c,
                    tc.tile_pool(name="c_out", bufs=3) as out_pool,
                    tc.tile_pool(name="c_tmp", bufs=3) as tmp_pool,
                    tc.tile_pool(name="c_small", bufs=4) as small_pool,
                ):
                    av_chunks = [
                        (c0, min(c0 + 512, dp1)) for c0 in range(0, dp1, 512)
                    ]
                    for i in range(nt):
                        ps1 = ps_c.tile([128, dp1], F32, tag="ps1")
                        ps2 = ps_c.tile([128, dp1], F32, tag="ps2")
                        for k in range(nt):
                            # F block (gi=i, lj=k) is the term1 lhsT
                            lhs_f = f_sb[:, i, _ts(k, 128)]
                            lhs_e = e_sb[:, k, _ts(i, 128)]
                            st = k == 0
                            sp = k == nt - 1
                            for c0, c1 in av_chunks:
                                nc.tensor.matmul(
                                    ps1[:, c0:c1], lhs_f, ln_sb[:, k, c0:c1],
                                    start=st, stop=sp,
                                )
                            for c0, c1 in av_chunks:
                                nc.tensor.matmul(
                                    ps2[:, c0:c1], lhs_e, gn_sb[:, k, c0:c1],
                                    start=st, stop=sp,
                                )
                        # epilogue: out = 0.5*(ps1[:, :d]/Z1 + ps2[:, :d]/Z2)
                        r1 = small_pool.tile([128, 1], F32, tag="r1")
                        r2 = small_pool.tile([128, 1], F32, tag="r2")
                        nc.vector.reciprocal(r1[:], ps1[:, d:dp1])
                        nc.vector.reciprocal(r2[:], ps2[:, d:dp1])
                        nc.vector.tensor_scalar_mul(r1[:], r1[:], 0.5)
                        nc.vector.tensor_scalar_mul(r2[:], r2[:], 0.5)
                        t1 = tmp_pool.tile([128, d], F32)
                        nc.scalar.activation(
                            t1[:], ps1[:, 0:d],
                            mybir.ActivationFunctionType.Copy,
                            scale=r1[:],
                        )
                        out_t = out_pool.tile([128, d], F32)
                        nc.vector.scalar_tensor_tensor(
                            out_t[:], ps2[:, 0:d], r2[:], t1[:],
                            op0=mybir.AluOpType.mult,
                            op1=mybir.AluOpType.add,
                        )
                        nc.sync.dma_start(out_d[_ts(i, 128), :], out_t[:])

    nc.compile()
    return nc


_NC_CACHE = {}


def get_nc(n=N, d=D):
    key = (n, d)
    if key not in _NC_CACHE:
        _NC_CACHE[key] = build_nc(n, d)
    return _NC_CACHE[key]


def host_prep(global_embedding, local_embedding):
    """Build the 8 per-core input maps from full [B, N, D] fp32 inputs."""
    g = np.asarray(global_embedding, dtype=np.float32)
    l = np.asarray(local_embedding, dtype=np.float32)
    b, n, d = g.shape
    nt, kd, nchunk = n // 128, d // 128, n // 512
    ident = np.eye(128, dtype=ml_dtypes.bfloat16)
    ones = np.ones((n, 1), np.float32)
    in_maps = []
    for i in range(b):
        # gtt[gi, p, k, c] = g[i][gi*128 + c, k*128 + p]
        gtt = np.ascontiguousarray(
            g[i].reshape(nt, 128, kd, 128).transpose(0, 3, 2, 1)
        )
        # ltt[j, p, k, c] = l[i][j*512 + c, k*128 + p]
        ltt = np.ascontiguousarray(
            l[i].reshape(nchunk, 512, kd, 128).transpose(0, 3, 2, 1)
        )
        in_maps.append(
            {
                "gtt": gtt,
                "ltt": ltt,
                "gn": np.concatenate([g[i], ones], axis=1).astype(
                    ml_dtypes.bfloat16
                ),
                "ln": np.concatenate([l[i], ones], axis=1).astype(
                    ml_dtypes.bfloat16
                ),
                "ident": ident,
            }
        )
    return in_maps


def kernel(global_embedding, local_embedding):
    g = np.asarray(global_embedding, dtype=np.float32)
    b, n, d = g.shape
    nc = get_nc(n, d)
    in_maps = host_prep(global_embedding, local_embedding)
    res = run_bass_kernel_spmd(nc, in_maps, list(range(N_CORES)))
    return np.stack([res.results[i]["out"] for i in range(b)]).astype(np.float32)


# revision 22
# speedup vs baseline: 1.0724x; 1.0203x over previous
"""Bi-directional attention fusion kernel for Trainium2 (8 NeuronCores).

Computes, per batch b (fully data-parallel, one batch per core):
    S       = g @ l.T                                  # [N, N]
    out     = 0.5 * (softmax_rows(S) @ l + softmax_rows(S.T) @ g)

Key algebraic trick: with a *global* stabilization constant c,
    E = exp(S - c)   (layout [g, l])
    F = exp(S.T - c) = E.T  (layout [l, g])
and the two attention terms become plain matmuls where E and F are the
pre-transposed (lhsT) operands directly:
    term1[g, d] = sum_l E[g, l] * l_emb[l, d]  =  (F as lhsT).T @ [l_emb | 1]
    term2[l, d] = sum_g E[g, l] * g_emb[g, d]  =  (E as lhsT).T @ [g_emb | 1]
The appended ones-column yields the softmax normalizers for free.

Schedule per core:
    A: S row-tiles via fp32r matmuls (1 cyc/row), exp via ACT -> E (bf16).
       Kept as one unbroken matmul stream (interleaving transposes into it
       measurably slows the fp32r pipeline).
    C: per out-tile i (all bf16 on PE):
       PE-transpose E row i+1 -> F row (rotating buffer, pipelined one
       ahead), term1 AV matmuls, term2 AV matmuls, ones-column
       normalizers, normalize/average epilogue.
"""

import numpy as np
import ml_dtypes

import concourse.bass as bass
import concourse.tile as tile
from concourse import bacc, mybir
from concourse.bass_utils import run_bass_kernel_spmd

BF16 = mybir.dt.bfloat16
F32 = mybir.dt.float32
F32R = mybir.dt.float32r

B = 8
N = 2048
D = 768
C_STAB = 116.0  # global softmax shift; logits are N(0, sqrt(768)) -> max ~111

N_CORES = 8
N_WARMUP_MM = 60  # dummy matmuls holding the PE HAM warm during input DMA
_ts = bass.ts


def build_nc(n=N, d=D, c_stab=C_STAB):
    """Build the per-core Bass program (identical on all cores)."""
    nt = n // 128  # row tiles
    kd = d // 128  # contraction tiles over the embedding dim
    dp1 = d + 1  # ones column appended
    nh = n // 2
    nchunk = n // 512  # 512-wide column chunks of S / lt

    nc = bacc.Bacc(None, target_bir_lowering=False)

    # host-pretiled inputs (every DMA is contiguous per partition row):
    #   gtt[gi][p, k, c] = g[gi*128+c, k*128+p]
    #   ltt[j][p, k, c]  = l[j*512+c, k*128+p]
    gt_d = nc.dram_tensor("gtt", [nt, 128, kd, 128], F32R, kind="ExternalInput")
    lt_d = nc.dram_tensor("ltt", [nchunk, 128, kd, 512], F32R, kind="ExternalInput")
    gn_d = nc.dram_tensor("gn", [n, dp1], BF16, kind="ExternalInput")
    ln_d = nc.dram_tensor("ln", [n, dp1], BF16, kind="ExternalInput")
    id_d = nc.dram_tensor("ident", [128, 128], BF16, kind="ExternalInput")
    out_d = nc.dram_tensor("out", [n, d], F32, kind="ExternalOutput")

    gn_r = gn_d[:].rearrange("(i p) d -> p i d", p=128)  # [128, nt, dp1]
    ln_r = ln_d[:].rearrange("(i p) d -> p i d", p=128)

    with tile.TileContext(nc) as tc:
        with (
            tc.tile_pool(name="const", bufs=1) as const_pool,
            tc.tile_pool(name="e", bufs=1) as e_pool,
            tc.tile_pool(name="nat_in", bufs=1) as nat_pool,
            tc.tile_pool(name="a_lt", bufs=1) as lt_pool,
            tc.tile_pool(name="a_gt", bufs=2) as gt_pool,
            tc.tile_pool(name="frow", bufs=2) as frow_pool,
            tc.tile_pool(name="c_out", bufs=3) as out_pool,
            tc.tile_pool(name="c_tmp", bufs=3) as tmp_pool,
            tc.tile_pool(name="c_small", bufs=4) as small_pool,
        ):
            ident = const_pool.tile([128, 128], BF16)
            nc.sync.dma_start(ident[:], id_d[:])
            biasc = const_pool.tile([128, 1], F32)
            nc.vector.memset(biasc[:], -c_stab)
            e_sb = e_pool.tile([128, nt, n], BF16)  # E[g, l]
            ln_sb = nat_pool.tile([128, nt, dp1], BF16)
            gn_sb = nat_pool.tile([128, nt, dp1], BF16)

            # ---- Stage A: E = exp(g @ l.T - c) ----
            with tc.tile_pool(name="ps_a", bufs=2, space="PSUM") as ps_a:
                lt_sb = lt_pool.tile([128, kd, n], F32R)
                nc.sync.dma_start(lt_sb[:, :, 0:512], lt_d[0])

                # HAM warm-up on a ps-tag slot (results are dummies)
                wps = ps_a.tile([128, n], F32, tag="ps")
                for _ in range(N_WARMUP_MM):
                    nc.tensor.matmul(wps[:, 0:128], ident[:], ident[:])

                for gi in range(nt):
                    gts = gt_pool.tile([128, kd, 128], F32R, tag="gts")
                    nc.sync.dma_start(gts[:], gt_d[gi])
                    if gi == 0:
                        for j in range(1, nchunk):
                            nc.sync.dma_start(lt_sb[:, :, _ts(j, 512)], lt_d[j])
                    if gi == 1:
                        # stage C inputs stream in behind the lt/gt loads
                        for cc in range(0, nt, 4):
                            nc.sync.dma_start(
                                ln_sb[:, cc : cc + 4, :], ln_r[:, cc : cc + 4, :]
                            )
                        for cc in range(0, nt, 4):
                            nc.sync.dma_start(
                                gn_sb[:, cc : cc + 4, :], gn_r[:, cc : cc + 4, :]
                            )
                    ps = ps_a.tile([128, n], F32, tag="ps")
                    for j in range(nchunk):
                        for k in range(kd):
                            nc.tensor.matmul(
                                ps[:, _ts(j, 512)],
                                gts[:, k, :],
                                lt_sb[:, k, _ts(j, 512)],
                                start=(k == 0),
                                stop=(k == kd - 1),
                            )
                    for j in range(nchunk):
                        nc.scalar.activation(
                            e_sb[:, gi, _ts(j, 512)],
                            ps[:, _ts(j, 512)],
                            mybir.ActivationFunctionType.Exp,
                            bias=biasc[:],
                        )

            # ---- Stage C: transpose + AV matmuls + normalize, per out-tile ----
            with (
                tc.tile_pool(name="c_pb", bufs=2, space="PSUM") as ps_t,
                tc.tile_pool(name="c_ps", bufs=1, space="PSUM") as ps_c,
            ):
                av_chunks = [(c0, min(c0 + 512, dp1)) for c0 in range(0, dp1, 512)]
                frows = {}

                def emit_transpose(i):
                    # F row i: block (i, lj) = transpose of e_sb[:, i, lj*128]
                    pb = ps_t.tile([128, n], BF16, tag="pb")
                    for lj in range(nt):
                        nc.tensor.transpose(
                            pb[:, _ts(lj, 128)],
                            e_sb[:, i, _ts(lj, 128)],
                            ident[:],
                        )
                    fr = frow_pool.tile([128, n], BF16, tag="frow")
                    nc.scalar.copy(fr[:, 0:nh], pb[:, 0:nh])
                    nc.vector.tensor_copy(fr[:, nh:n], pb[:, nh:n])
                    frows[i] = fr

                emit_transpose(0)
                for i in range(nt):
                    if i + 1 < nt:
                        emit_transpose(i + 1)
                    fr = frows.pop(i)
                    ps1 = ps_c.tile([128, dp1], F32, tag="ps1")
                    ps2 = ps_c.tile([128, dp1], F32, tag="ps2")
                    for k in range(nt):
                        lhs_f = fr[:, _ts(k, 128)]
                        for c0, c1 in av_chunks:
                            nc.tensor.matmul(
                                ps1[:, c0:c1], lhs_f, ln_sb[:, k, c0:c1],
                                start=(k == 0), stop=(k == nt - 1),
                            )
                    for k in range(nt):
                        lhs_e = e_sb[:, k, _ts(i, 128)]
                        for c0, c1 in av_chunks:
                            nc.tensor.matmul(
                                ps2[:, c0:c1], lhs_e, gn_sb[:, k, c0:c1],
                                start=(k == 0), stop=(k == nt - 1),
                            )
                    # epilogue: out = 0.5*(ps1[:, :d]/Z1 + ps2[:, :d]/Z2)
                    r1 = small_pool.tile([128, 1], F32, tag="r1")
                    r2 = small_pool.tile([128, 1], F32, tag="r2")
                    nc.vector.reciprocal(r1[:], ps1[:, d:dp1])
                    nc.vector.reciprocal(r2[:], ps2[:, d:dp1])
                    nc.vector.tensor_scalar_mul(r1[:], r1[:], 0.5)
                    nc.vector.tensor_scalar_mul(r2[:], r2[:], 0.5)
                    t1 = tmp_pool.tile([128, d], F32)
                    nc.scalar.activation(
                        t1[:], ps1[:, 0:d],
                        mybir.ActivationFunctionType.Copy,
                        scale=r1[:],
                    )
                    out_t = out_pool.tile([128, d], F32)
                    nc.vector.scalar_tensor_tensor(
                        out_t[:], ps2[:, 0:d], r2[:], t1[:],
                        op0=mybir.AluOpType.mult,
                        op1=mybir.AluOpType.add,
                    )
                    nc.sync.dma_start(out_d[_ts(i, 128), :], out_t[:])

    nc.compile()
    return nc


_NC_CACHE = {}


def get_nc(n=N, d=D):
    key = (n, d)
    if key not in _NC_CACHE:
        _NC_CACHE[key] = build_nc(n, d)
    return _NC_CACHE[key]


def host_prep(global_embedding, local_embedding):
    """Build the 8 per-core input maps from full [B, N, D] fp32 inputs."""
    g = np.asarray(global_embedding, dtype=np.float32)
    l = np.asarray(local_embedding, dtype=np.float32)
    b, n, d = g.shape
    nt, kd, nchunk = n // 128, d // 128, n // 512
    ident = np.eye(128, dtype=ml_dtypes.bfloat16)
    ones = np.ones((n, 1), np.float32)
    in_maps = []
    for i in range(b):
        # gtt[gi, p, k, c] = g[i][gi*128 + c, k*128 + p]
        gtt = np.ascontiguousarray(
            g[i].reshape(nt, 128, kd, 128).transpose(0, 3, 2, 1)
        )
        # ltt[j, p, k, c] = l[i][j*512 + c, k*128 + p]
        ltt = np.ascontiguousarray(
            l[i].reshape(nchunk, 512, kd, 128).transpose(0, 3, 2, 1)
        )
        in_maps.append(
            {
                "gtt": gtt,
                "ltt": ltt,
                "gn": np.concatenate([g[i], ones], axis=1).astype(
                    ml_dtypes.bfloat16
                ),
                "ln": np.concatenate([l[i], ones], axis=1).astype(
                    ml_dtypes.bfloat16
                ),
                "ident": ident,
            }
        )
    return in_maps


def kernel(global_embedding, local_embedding):
    g = np.asarray(global_embedding, dtype=np.float32)
    b, n, d = g.shape
    nc = get_nc(n, d)
    in_maps = host_prep(global_embedding, local_embedding)
    res = run_bass_kernel_spmd(nc, in_maps, list(range(N_CORES)))
    return np.stack([res.results[i]["out"] for i in range(b)]).astype(np.float32)


# revision 23
# speedup vs baseline: 1.1081x; 1.0333x over previous
"""Bi-directional attention fusion kernel for Trainium2 (8 NeuronCores).

Computes, per batch b (fully data-parallel, one batch per core):
    S       = g @ l.T                                  # [N, N]
    out     = 0.5 * (softmax_rows(S) @ l + softmax_rows(S.T) @ g)

Key algebraic trick: with a *global* stabilization constant c,
    E = exp(S - c)   (layout [g, l])
    F = exp(S.T - c) = E.T  (layout [l, g])
and the two attention terms become plain matmuls where E and F are the
pre-transposed (lhsT) operands directly:
    term1[g, d] = sum_l E[g, l] * l_emb[l, d]  =  (F as lhsT).T @ [l_emb | 1]
    term2[l, d] = sum_g E[g, l] * g_emb[g, d]  =  (E as lhsT).T @ [g_emb | 1]
The appended ones-column yields the softmax normalizers for free.

Schedule per core:
    A: S row-tiles via fp32r matmuls (1 cyc/row), exp via ACT -> E (bf16).
       Kept as one unbroken matmul stream (interleaving transposes into it
       measurably slows the fp32r pipeline).
    C: per out-tile i (all bf16 on PE):
       PE-transpose E row i+1 -> F row (rotating buffer, pipelined one
       ahead), term1 AV matmuls, term2 AV matmuls, ones-column
       normalizers, normalize/average epilogue.
"""

import numpy as np
import ml_dtypes

import concourse.bass as bass
import concourse.tile as tile
from concourse import bacc, mybir
from concourse.bass_utils import run_bass_kernel_spmd

BF16 = mybir.dt.bfloat16
F32 = mybir.dt.float32
F32R = mybir.dt.float32r

B = 8
N = 2048
D = 768
C_STAB = 116.0  # global softmax shift; logits are N(0, sqrt(768)) -> max ~111

N_CORES = 8
N_WARMUP_MM = 60  # dummy matmuls holding the PE HAM warm during input DMA
_ts = bass.ts


def build_nc(n=N, d=D, c_stab=C_STAB):
    """Build the per-core Bass program (identical on all cores)."""
    nt = n // 128  # row tiles
    kd = d // 128  # contraction tiles over the embedding dim
    dp1 = d + 1  # ones column appended
    nh = n // 2
    nchunk = n // 512  # 512-wide column chunks of S / lt

    nc = bacc.Bacc(None, target_bir_lowering=False)

    # host-pretiled inputs (every DMA is contiguous per partition row):
    #   gtt[gi][p, k, c] = g[gi*128+c, k*128+p]
    #   ltt[j][p, k, c]  = l[j*512+c, k*128+p]
    gt_d = nc.dram_tensor("gtt", [nt, 128, kd, 128], F32R, kind="ExternalInput")
    lt_d = nc.dram_tensor("ltt", [nchunk, 128, kd, 512], F32R, kind="ExternalInput")
    gn_d = nc.dram_tensor("gn", [n, dp1], BF16, kind="ExternalInput")
    ln_d = nc.dram_tensor("ln", [n, dp1], BF16, kind="ExternalInput")
    id_d = nc.dram_tensor("ident", [128, 128], BF16, kind="ExternalInput")
    out_d = nc.dram_tensor("out", [n, d], F32, kind="ExternalOutput")

    gn_r = gn_d[:].rearrange("(i p) d -> p i d", p=128)  # [128, nt, dp1]
    ln_r = ln_d[:].rearrange("(i p) d -> p i d", p=128)

    with tile.TileContext(nc) as tc:
        with (
            tc.tile_pool(name="const", bufs=1) as const_pool,
            tc.tile_pool(name="e", bufs=1) as e_pool,
            tc.tile_pool(name="nat_in", bufs=1) as nat_pool,
            tc.tile_pool(name="a_lt", bufs=1) as lt_pool,
            tc.tile_pool(name="a_gt", bufs=2) as gt_pool,
            tc.tile_pool(name="frow", bufs=2) as frow_pool,
            tc.tile_pool(name="c_out", bufs=3) as out_pool,
            tc.tile_pool(name="c_tmp", bufs=3) as tmp_pool,
            tc.tile_pool(name="c_small", bufs=4) as small_pool,
        ):
            ident = const_pool.tile([128, 128], BF16)
            nc.sync.dma_start(ident[:], id_d[:])
            biasc = const_pool.tile([128, 1], F32)
            nc.vector.memset(biasc[:], -c_stab)
            e_sb = e_pool.tile([128, nt, n], BF16)  # E[g, l]
            ln_sb = nat_pool.tile([128, nt, dp1], BF16)
            gn_sb = nat_pool.tile([128, nt, dp1], BF16)

            # ---- Stage A: E = exp(g @ l.T - c) ----
            with tc.tile_pool(name="ps_a", bufs=2, space="PSUM") as ps_a:
                lt_sb = lt_pool.tile([128, kd, n], F32R)
                nc.sync.dma_start(lt_sb[:, :, 0:512], lt_d[0])

                # HAM warm-up on a ps-tag slot (results are dummies)
                wps = ps_a.tile([128, n], F32, tag="ps")
                for _ in range(N_WARMUP_MM):
                    nc.tensor.matmul(wps[:, 0:128], ident[:], ident[:])

                for gi in range(nt):
                    gts = gt_pool.tile([128, kd, 128], F32R, tag="gts")
                    nc.sync.dma_start(gts[:], gt_d[gi])
                    if gi == 0:
                        for j in range(1, nchunk):
                            nc.sync.dma_start(lt_sb[:, :, _ts(j, 512)], lt_d[j])
                    if gi == 8:
                        # stage C inputs stream in behind the lt/gt loads
                        for cc in range(0, nt, 4):
                            nc.sync.dma_start(
                                ln_sb[:, cc : cc + 4, :], ln_r[:, cc : cc + 4, :]
                            )
                        for cc in range(0, nt, 4):
                            nc.sync.dma_start(
                                gn_sb[:, cc : cc + 4, :], gn_r[:, cc : cc + 4, :]
                            )
                    ps = ps_a.tile([128, n], F32, tag="ps")
                    for j in range(nchunk):
                        for k in range(kd):
                            nc.tensor.matmul(
                                ps[:, _ts(j, 512)],
                                gts[:, k, :],
                                lt_sb[:, k, _ts(j, 512)],
                                start=(k == 0),
                                stop=(k == kd - 1),
                            )
                    for j in range(nchunk):
                        nc.scalar.activation(
                            e_sb[:, gi, _ts(j, 512)],
                            ps[:, _ts(j, 512)],
                            mybir.ActivationFunctionType.Exp,
                            bias=biasc[:],
                        )

            # ---- Stage C: transpose + AV matmuls + normalize, per out-tile ----
            with (
                tc.tile_pool(name="c_pb", bufs=2, space="PSUM") as ps_t,
                tc.tile_pool(name="c_ps", bufs=1, space="PSUM") as ps_c,
            ):
                av_chunks = [(c0, min(c0 + 512, dp1)) for c0 in range(0, dp1, 512)]
                frows = {}

                def emit_transpose(i):
                    # F row i: block (i, lj) = transpose of e_sb[:, i, lj*128]
                    pb = ps_t.tile([128, n], BF16, tag="pb")
                    for lj in range(nt):
                        nc.tensor.transpose(
                            pb[:, _ts(lj, 128)],
                            e_sb[:, i, _ts(lj, 128)],
                            ident[:],
                        )
                    fr = frow_pool.tile([128, n], BF16, tag="frow")
                    nc.scalar.copy(fr[:, 0:nh], pb[:, 0:nh])
                    nc.vector.tensor_copy(fr[:, nh:n], pb[:, nh:n])
                    frows[i] = fr

                emit_transpose(0)
                for i in range(nt):
                    if i + 1 < nt:
                        emit_transpose(i + 1)
                    fr = frows.pop(i)
                    ps1 = ps_c.tile([128, dp1], F32, tag="ps1")
                    ps2 = ps_c.tile([128, dp1], F32, tag="ps2")
                    for k in range(nt):
                        lhs_f = fr[:, _ts(k, 128)]
                        for c0, c1 in av_chunks:
                            nc.tensor.matmul(
                                ps1[:, c0:c1], lhs_f, ln_sb[:, k, c0:c1],
                                start=(k == 0), stop=(k == nt - 1),
                            )
                    for k in range(nt):
                        lhs_e = e_sb[:, k, _ts(i, 128)]
                        for c0, c1 in av_chunks:
                            nc.tensor.matmul(
                                ps2[:, c0:c1], lhs_e, gn_sb[:, k, c0:c1],
                                start=(k == 0), stop=(k == nt - 1),
                            )
                    # epilogue: out = 0.5*(ps1[:, :d]/Z1 + ps2[:, :d]/Z2)
                    r1 = small_pool.tile([128, 1], F32, tag="r1")
                    r2 = small_pool.tile([128, 1], F32, tag="r2")
                    nc.vector.reciprocal(r1[:], ps1[:, d:dp1])
                    nc.vector.reciprocal(r2[:], ps2[:, d:dp1])
                    nc.vector.tensor_scalar_mul(r1[:], r1[:], 0.5)
                    nc.vector.tensor_scalar_mul(r2[:], r2[:], 0.5)
                    t1 = tmp_pool.tile([128, d], F32)
                    nc.scalar.activation(
                        t1[:], ps1[:, 0:d],
                        mybir.ActivationFunctionType.Copy,
                        scale=r1[:],
                    )
                    out_t = out_pool.tile([128, d], F32)
                    nc.vector.scalar_tensor_tensor(
                        out_t[:], ps2[:, 0:d], r2[:], t1[:],
                        op0=mybir.AluOpType.mult,
                        op1=mybir.AluOpType.add,
                    )
                    nc.sync.dma_start(out_d[_ts(i, 128), :], out_t[:])

    nc.compile()
    return nc


_NC_CACHE = {}


def get_nc(n=N, d=D):
    key = (n, d)
    if key not in _NC_CACHE:
        _NC_CACHE[key] = build_nc(n, d)
    return _NC_CACHE[key]


def host_prep(global_embedding, local_embedding):
    """Build the 8 per-core input maps from full [B, N, D] fp32 inputs."""
    g = np.asarray(global_embedding, dtype=np.float32)
    l = np.asarray(local_embedding, dtype=np.float32)
    b, n, d = g.shape
    nt, kd, nchunk = n // 128, d // 128, n // 512
    ident = np.eye(128, dtype=ml_dtypes.bfloat16)
    ones = np.ones((n, 1), np.float32)
    in_maps = []
    for i in range(b):
        # gtt[gi, p, k, c] = g[i][gi*128 + c, k*128 + p]
        gtt = np.ascontiguousarray(
            g[i].reshape(nt, 128, kd, 128).transpose(0, 3, 2, 1)
        )
        # ltt[j, p, k, c] = l[i][j*512 + c, k*128 + p]
        ltt = np.ascontiguousarray(
            l[i].reshape(nchunk, 512, kd, 128).transpose(0, 3, 2, 1)
        )
        in_maps.append(
            {
                "gtt": gtt,
                "ltt": ltt,
                "gn": np.concatenate([g[i], ones], axis=1).astype(
                    ml_dtypes.bfloat16
                ),
                "ln": np.concatenate([l[i], ones], axis=1).astype(
                    ml_dtypes.bfloat16
                ),
                "ident": ident,
            }
        )
    return in_maps


def kernel(global_embedding, local_embedding):
    g = np.asarray(global_embedding, dtype=np.float32)
    b, n, d = g.shape
    nc = get_nc(n, d)
    in_maps = host_prep(global_embedding, local_embedding)
    res = run_bass_kernel_spmd(nc, in_maps, list(range(N_CORES)))
    return np.stack([res.results[i]["out"] for i in range(b)]).astype(np.float32)


# revision 34
# speedup vs baseline: 1.1616x; 1.0482x over previous
"""Bi-directional attention fusion kernel for Trainium2 (8 NeuronCores).

Computes, per batch b (fully data-parallel, one batch per core):
    S       = g @ l.T                                  # [N, N]
    out     = 0.5 * (softmax_rows(S) @ l + softmax_rows(S.T) @ g)

Key algebraic trick: with a *global* stabilization constant c,
    E = exp(S - c)   (layout [g, l])
    F = exp(S.T - c) = E.T  (layout [l, g])
and the two attention terms become plain matmuls where E and F are the
pre-transposed (lhsT) operands directly:
    term1[g, d] = sum_l E[g, l] * l_emb[l, d]  =  (F as lhsT).T @ [l_emb | 1]
    term2[l, d] = sum_g E[g, l] * g_emb[g, d]  =  (E as lhsT).T @ [g_emb | 1]
The appended ones-column yields the softmax normalizers for free.

Schedule per core:
    A: S row-tiles via fp32r matmuls (1 cyc/row), exp via ACT -> E (bf16).
       Kept as one unbroken matmul stream (interleaving transposes into it
       measurably slows the fp32r pipeline).
    C: per out-tile i (all bf16 on PE):
       PE-transpose E row i+1 -> F row (rotating buffer, pipelined one
       ahead), term1 AV matmuls, term2 AV matmuls, ones-column
       normalizers, normalize/average epilogue.
"""

import numpy as np
import ml_dtypes

import concourse.bass as bass
import concourse.tile as tile
from concourse import bacc, mybir
from concourse.bass_utils import run_bass_kernel_spmd

BF16 = mybir.dt.bfloat16
F32 = mybir.dt.float32
F32R = mybir.dt.float32r

B = 8
N = 2048
D = 768
C_STAB = 116.0  # global softmax shift; logits are N(0, sqrt(768)) -> max ~111

N_CORES = 8
N_WARMUP_MM = 60  # dummy matmuls holding the PE HAM warm during input DMA
_ts = bass.ts


def build_nc(n=N, d=D, c_stab=C_STAB):
    """Build the per-core Bass program (identical on all cores)."""
    nt = n // 128  # row tiles
    kd = d // 128  # contraction tiles over the embedding dim
    dp1 = d + 1  # ones column appended
    nh = n // 2
    nchunk = n // 512  # 512-wide column chunks of S / lt

    nc = bacc.Bacc(None, target_bir_lowering=False)

    # host-pretiled inputs (every DMA is contiguous per partition row):
    #   gtt[gi][p, k, c] = g[gi*128+c, k*128+p]
    #   ltt[j][p, k, c]  = l[j*512+c, k*128+p]
    gt_d = nc.dram_tensor("gtt", [nt, 128, kd, 128], F32R, kind="ExternalInput")
    lt_d = nc.dram_tensor("ltt", [128, nchunk, kd, 512], F32R, kind="ExternalInput")
    gn_d = nc.dram_tensor("gnt", [128, nt, dp1], BF16, kind="ExternalInput")
    ln_d = nc.dram_tensor("lnt", [128, nt, dp1], BF16, kind="ExternalInput")
    id_d = nc.dram_tensor("ident", [128, 128], BF16, kind="ExternalInput")
    out_d = nc.dram_tensor("out", [n, d], F32, kind="ExternalOutput")

    with tile.TileContext(nc) as tc:
        with (
            tc.tile_pool(name="const", bufs=1) as const_pool,
            tc.tile_pool(name="e", bufs=1) as e_pool,
            tc.tile_pool(name="nat_in", bufs=1) as nat_pool,
            tc.tile_pool(name="a_lt", bufs=1) as lt_pool,
            tc.tile_pool(name="a_gt", bufs=5) as gt_pool,
            tc.tile_pool(name="frow", bufs=4) as frow_pool,
            tc.tile_pool(name="c_out", bufs=2) as out_pool,
            tc.tile_pool(name="c_tmp", bufs=2) as tmp_pool,
            tc.tile_pool(name="c_small", bufs=4) as small_pool,
        ):
            ident = const_pool.tile([128, 128], BF16)
            nc.sync.dma_start(ident[:], id_d[:])
            biasc = const_pool.tile([128, 1], F32)
            nc.vector.memset(biasc[:], -c_stab)
            e_sb = e_pool.tile([128, nt, n], BF16)  # E[g, l]
            ln_sb = nat_pool.tile([128, nt, dp1], BF16)
            gn_sb = nat_pool.tile([128, nt, dp1], BF16)

            # ---- Stage A: E = exp(g @ l.T - c) ----
            # Half-width PSUM tiles (2 banks, bufs=4): the first three row
            # tiles interleave their column chunks so PE compute covers the
            # streaming lt chunk arrivals; later row tiles run chunk-major.
            with tc.tile_pool(name="ps_a", bufs=4, space="PSUM") as ps_a:
                lt_sb = lt_pool.tile([128, nchunk, kd, 512], F32R)
                nc.sync.dma_start(lt_sb[:, 0], lt_d[:, 0])

                n_ilv = min(4, nt)

                def emit_mm(ps_half, jj, gts, j):
                    for k in range(kd):
                        nc.tensor.matmul(
                            ps_half[:, _ts(jj, 512)],
                            gts[:, k, :],
                            lt_sb[:, j, k, :],
                            start=(k == 0),
                            stop=(k == kd - 1),
                        )

                def emit_exp(ps_half, gi, jbase):
                    for jj in range(nchunk // 2):
                        nc.scalar.activation(
                            e_sb[:, gi, (jbase + jj) * 512 : (jbase + jj + 1) * 512],
                            ps_half[:, _ts(jj, 512)],
                            mybir.ActivationFunctionType.Exp,
                            bias=biasc[:],
                        )

                gts_i = []
                for gi in range(n_ilv):
                    gts = gt_pool.tile([128, kd, 128], F32R, tag="gts")
                    nc.sync.dma_start(gts[:], gt_d[gi])
                    gts_i.append(gts)
                for j in range(1, nchunk):
                    nc.sync.dma_start(lt_sb[:, j], lt_d[:, j])

                psa_i = [
                    ps_a.tile([128, nh], F32, tag="ps", name=f"psa{gi}")
                    for gi in range(n_ilv)
                ]
                # HAM warm-up dummies into psa_i[0]; the first real matmul
                # (start=True) clears the bank, so results are harmless
                for _ in range(N_WARMUP_MM):
                    nc.tensor.matmul(psa_i[0][:, 0:128], ident[:], ident[:])
                for j in range(nchunk // 2):
                    for gi in range(n_ilv):
                        emit_mm(psa_i[gi], j, gts_i[gi], j)
                for gi in range(n_ilv):
                    emit_exp(psa_i[gi], gi, 0)
                psb_i = [
                    ps_a.tile([128, nh], F32, tag="ps", name=f"psb{gi}")
                    for gi in range(n_ilv)
                ]
                for j in range(nchunk // 2, nchunk):
                    for gi in range(n_ilv):
                        emit_mm(psb_i[gi], j - nchunk // 2, gts_i[gi], j)
                for gi in range(n_ilv):
                    emit_exp(psb_i[gi], gi, nchunk // 2)

                for gi in range(n_ilv, nt):
                    gts = gt_pool.tile([128, kd, 128], F32R, tag="gts")
                    nc.sync.dma_start(gts[:], gt_d[gi])
                    if gi == min(8, nt - 1):
                        # stage C inputs stream in behind the lt/gt loads
                        half = nt // 2
                        nc.sync.dma_start(ln_sb[:, 0:half, :], ln_d[:, 0:half, :])
                        nc.sync.dma_start(ln_sb[:, half:, :], ln_d[:, half:, :])
                        nc.sync.dma_start(gn_sb[:, 0:half, :], gn_d[:, 0:half, :])
                        nc.sync.dma_start(gn_sb[:, half:, :], gn_d[:, half:, :])
                    psa = ps_a.tile([128, nh], F32, tag="ps")
                    for j in range(nchunk // 2):
                        emit_mm(psa, j, gts, j)
                    emit_exp(psa, gi, 0)
                    psb = ps_a.tile([128, nh], F32, tag="ps")
                    for j in range(nchunk // 2, nchunk):
                        emit_mm(psb, j - nchunk // 2, gts, j)
                    emit_exp(psb, gi, nchunk // 2)

            # ---- Stage C: transpose + AV matmuls + normalize, per out-tile ----
            with (
                tc.tile_pool(name="c_pb", bufs=2, space="PSUM") as ps_t,
                tc.tile_pool(name="c_ps", bufs=1, space="PSUM") as ps_c,
            ):
                av_chunks = [(c0, min(c0 + 512, dp1)) for c0 in range(0, dp1, 512)]
                frows = {}

                def emit_transpose(i):
                    # F row i: block (i, lj) = transpose of e_sb[:, i, lj*128]
                    pb = ps_t.tile([128, n], BF16, tag="pb")
                    for lj in range(nt):
                        nc.tensor.transpose(
                            pb[:, _ts(lj, 128)],
                            e_sb[:, i, _ts(lj, 128)],
                            ident[:],
                        )
                    fr = frow_pool.tile([128, n], BF16, tag="frow")
                    nc.scalar.copy(fr[:, 0:nh], pb[:, 0:nh])
                    nc.vector.tensor_copy(fr[:, nh:n], pb[:, nh:n])
                    frows[i] = fr

                emit_transpose(0)
                emit_transpose(1)
                for i in range(nt):
                    # batch two F-row transposes every other iteration to
                    # halve the matmul<->transpose mode transitions
                    if i % 2 == 0:
                        for ahead in (i + 2, i + 3):
                            if ahead < nt:
                                emit_transpose(ahead)
                    fr = frows.pop(i)
                    ps1 = ps_c.tile([128, dp1], F32, tag="ps1")
                    ps2 = ps_c.tile([128, dp1], F32, tag="ps2")
                    for k in range(nt):
                        lhs_f = fr[:, _ts(k, 128)]
                        for c0, c1 in av_chunks:
                            nc.tensor.matmul(
                                ps1[:, c0:c1], lhs_f, ln_sb[:, k, c0:c1],
                                start=(k == 0), stop=(k == nt - 1),
                            )
                    for k in range(nt):
                        lhs_e = e_sb[:, k, _ts(i, 128)]
                        for c0, c1 in av_chunks:
                            nc.tensor.matmul(
                                ps2[:, c0:c1], lhs_e, gn_sb[:, k, c0:c1],
                                start=(k == 0), stop=(k == nt - 1),
                            )
                    # epilogue: out = 0.5*(ps1[:, :d]/Z1 + ps2[:, :d]/Z2)
                    r1 = small_pool.tile([128, 1], F32, tag="r1")
                    r2 = small_pool.tile([128, 1], F32, tag="r2")
                    nc.vector.reciprocal(r1[:], ps1[:, d:dp1])
                    nc.vector.reciprocal(r2[:], ps2[:, d:dp1])
                    nc.vector.tensor_scalar_mul(r1[:], r1[:], 0.5)
                    nc.vector.tensor_scalar_mul(r2[:], r2[:], 0.5)
                    t1 = tmp_pool.tile([128, d], F32)
                    nc.scalar.activation(
                        t1[:], ps1[:, 0:d],
                        mybir.ActivationFunctionType.Copy,
                        scale=r1[:],
                    )
                    out_t = out_pool.tile([128, d], F32)
                    if i == nt - 1:
                        # split the final epilogue so the last output DMA
                        # starts as early as possible (shorter kernel tail)
                        hd = d // 2
                        for lo, hi in ((0, hd), (hd, d)):
                            nc.vector.scalar_tensor_tensor(
                                out_t[:, lo:hi], ps2[:, lo:hi], r2[:], t1[:, lo:hi],
                                op0=mybir.AluOpType.mult,
                                op1=mybir.AluOpType.add,
                            )
                            nc.sync.dma_start(
                                out_d[_ts(i, 128), lo:hi], out_t[:, lo:hi]
                            )
                    else:
                        nc.vector.scalar_tensor_tensor(
                            out_t[:], ps2[:, 0:d], r2[:], t1[:],
                            op0=mybir.AluOpType.mult,
                            op1=mybir.AluOpType.add,
                        )
                        nc.sync.dma_start(out_d[_ts(i, 128), :], out_t[:])

    nc.compile()
    return nc


_NC_CACHE = {}


def get_nc(n=N, d=D):
    key = (n, d)
    if key not in _NC_CACHE:
        _NC_CACHE[key] = build_nc(n, d)
    return _NC_CACHE[key]


def host_prep(global_embedding, local_embedding):
    """Build the 8 per-core input maps from full [B, N, D] fp32 inputs."""
    g = np.asarray(global_embedding, dtype=np.float32)
    l = np.asarray(local_embedding, dtype=np.float32)
    b, n, d = g.shape
    nt, kd, nchunk = n // 128, d // 128, n // 512
    ident = np.eye(128, dtype=ml_dtypes.bfloat16)
    ones = np.ones((n, 1), np.float32)
    in_maps = []
    for i in range(b):
        # gtt[gi, p, k, c] = g[i][gi*128 + c, k*128 + p]
        gtt = np.ascontiguousarray(
            g[i].reshape(nt, 128, kd, 128).transpose(0, 3, 2, 1)
        )
        # ltt[p, j, k, c] = l[i][j*512 + c, k*128 + p]
        ltt = np.ascontiguousarray(
            l[i].reshape(nchunk, 512, kd, 128).transpose(3, 0, 2, 1)
        )
        # gnt/lnt[p, i, :] = [emb | 1][i*128 + p, :]
        gnt = np.ascontiguousarray(
            np.concatenate([g[i], ones], axis=1)
            .astype(ml_dtypes.bfloat16)
            .reshape(nt, 128, d + 1)
            .transpose(1, 0, 2)
        )
        lnt = np.ascontiguousarray(
            np.concatenate([l[i], ones], axis=1)
            .astype(ml_dtypes.bfloat16)
            .reshape(nt, 128, d + 1)
            .transpose(1, 0, 2)
        )
        in_maps.append(
            {"gtt": gtt, "ltt": ltt, "gnt": gnt, "lnt": lnt, "ident": ident}
        )
    return in_maps


def kernel(global_embedding, local_embedding):
    g = np.asarray(global_embedding, dtype=np.float32)
    b, n, d = g.shape
    nc = get_nc(n, d)
    in_maps = host_prep(global_embedding, local_embedding)
    res = run_bass_kernel_spmd(nc, in_maps, list(range(N_CORES)))
    return np.stack([res.results[i]["out"] for i in range(b)]).astype(np.float32)
